# revision 8
# baseline (speedup 1.0000x reference)
"""Masked FFN kernel for trn2 (8 NeuronCores, SPMD data-parallel over rows).

Math: out = (gelu(x @ W1 + b1) @ W2 + b2) * mask  with masked-out rows exactly 0.

Strategy:
  - Host compacts the (B*T) rows down to the ~50% active ones (mask != 0),
    shards them evenly across 8 cores, pads per-core row count R to a
    block-friendly capacity.
  - All matmul operands are bf16 (fp32 PSUM accumulation, fp32 bias/act):
    halves DMA + SBUF traffic at the same PE rate; rel err ~4e-3.
  - Device computes the FFN on compacted rows only, in transposed layout:
      mm1: H^T[f, r] = sum_d W1[d, f] * X^T[d, r]   (W1 tile stationary)
      gelu+b1 fused on ScalarE (PSUM -> SBUF, bf16 out)
      mm2: Y^T[o, r] = sum_f W2[f, o] * H^T[f, r]   (W2 tile stationary)
      +b2 fused on ScalarE (PSUM -> SBUF fp32), DMA out.
  - Row blocks [256, ~512, ~512]; phase A-0 runs the small block as soon as
    ~1MB (x block 0 + first W1 group) lands; W1 streams on the same HWDGE
    ring right behind x block 0; x blocks 1-2 + biases ride gpsimd; W2
    triggers are interleaved into early phase A-12 on the scalar ring.
  - PE warm-up matmuls on a memset tile cover the DVFS ramp during the
    initial DMA wait; the last output tile finishes as three sequential
    accumulation groups so its drains overlap compute.
"""

import numpy as np
import ml_dtypes

import concourse.tile as tile
from concourse import bacc, mybir
from concourse import bass_utils

N_CORES = 8
D = 1024       # model dim
F = 4096       # ffn dim
DT = D // 128  # 8 d-tiles
FT = F // 128  # 32 f-tiles
OT = D // 128  # 8 output tiles
WG = 2                 # W1 f-tiles per DMA group
NWG = FT // WG         # 16 groups
N_WARM = 10            # PE warm-up matmuls

F32 = mybir.dt.float32
BF16 = mybir.dt.bfloat16
NP_BF16 = ml_dtypes.bfloat16

_CACHE: dict = {}
LAST_RESULTS = None  # BassKernelResults of the most recent device run (for test harness)


def _ensure_trace_support():
    """If BASS_TRACE is set but the NTFF hook module is missing, install a
    local shim so run_bass_kernel_spmd's trace path works instead of crashing."""
    import os
    if not os.environ.get("BASS_TRACE"):
        return
    import sys, types
    try:
        import antenv.axon_hooks  # noqa: F401
    except ImportError:
        mod = types.ModuleType("antenv.axon_hooks")
        mod._h = None
        mod.set_axon_ntff_profile_hook = lambda h: setattr(mod, "_h", h)
        mod.get_axon_ntff_profile_hook = lambda: mod._h
        sys.modules["antenv.axon_hooks"] = mod
        try:
            from trn_agent_boot.trn_boot import _ntff_profile_via_ctypes
            mod.set_axon_ntff_profile_hook(
                _ntff_profile_via_ctypes("/opt/axon/libaxon_pjrt.so")
            )
        except Exception:
            pass
    try:
        bass_utils.upload_artifacts  # noqa: B018
        bass_utils.upload_artifacts = lambda tmpdir: tmpdir
    except Exception:
        pass


def _blocks(rc: int):
    """Row blocks: 512 first (phase A-0 then consumes W1 slower than DMA
    delivers it -> no starvation), rest near-equal <= 512."""
    if rc <= 512:
        return [(0, rc)]
    rem = rc - 512
    n = -(-rem // 512)
    sizes = [rem // n + (1 if i < rem % n else 0) for i in range(n)]
    out = [(0, 512)]
    pos = 512
    for s in sizes:
        out.append((pos, s))
        pos += s
    return out


def _build(rc: int, nch: int):
    key = (rc, nch)
    if key in _CACHE:
        return _CACHE[key]

    nc = bacc.Bacc("TRN2", target_bir_lowering=False, debug=False, num_devices=N_CORES,
                   dynamic_dma_scratch_size=8192)
    blocks = _blocks(rc)
    nb = len(blocks)
    # one DRAM tensor per x block: per-partition fully contiguous -> static DMA
    x_d = [
        nc.dram_tensor(f"x{bi}", [nch, 128, DT * blk], BF16, kind="ExternalInput").ap()
        for bi, (b0, blk) in enumerate(blocks)
    ]
    w1_d = nc.dram_tensor("w1t", [NWG, 128, WG, DT, 128], BF16, kind="ExternalInput").ap()
    b1_d = nc.dram_tensor("b1m", [128, FT], F32, kind="ExternalInput").ap()
    w2_d = nc.dram_tensor("w2t", [OT, 128, FT, 128], BF16, kind="ExternalInput").ap()
    b2_d = nc.dram_tensor("b2m", [128, OT], F32, kind="ExternalInput").ap()
    yt_d = nc.dram_tensor("yt", [nch, OT, 128, rc], F32, kind="ExternalOutput").ap()

    gelu = mybir.ActivationFunctionType.Gelu_apprx_tanh
    ident = mybir.ActivationFunctionType.Identity

    with tile.TileContext(nc) as tc:
        with (
            tc.tile_pool(name="consts", bufs=1) as consts,
            tc.tile_pool(name="xpool", bufs=1) as xpool,
            tc.tile_pool(name="hpool", bufs=1) as hpool,
            tc.tile_pool(name="w1pool", bufs=1) as w1pool,
            tc.tile_pool(name="w2pool", bufs=3) as w2pool,
            tc.tile_pool(name="ypool", bufs=3) as ypool,
            tc.tile_pool(name="pspool", bufs=6, space="PSUM") as pspool,
            tc.tile_pool(name="warmps", bufs=1, space="PSUM") as warmps,
        ):
            b1_sb = consts.tile([128, FT], F32)
            b2_sb = consts.tile([128, OT], F32)

            # --- PE warm-up: ramp the clock while input DMAs are in flight ---
            warm_sb = consts.tile([128, 512], BF16)
            nc.vector.memset(warm_sb[:], 0.0)
            wps = warmps.tile([128, 512], F32)
            for _ in range(N_WARM):
                nc.tensor.matmul(wps[:], lhsT=warm_sb[:, :128], rhs=warm_sb[:],
                                 start=True, stop=True)

            w1_sb = []   # NWG group tiles, resident across chunks
            w2_sb = {}   # o_t -> rotating tile

            def w1ap(f_t, d_t):
                return w1_sb[f_t // WG][:, f_t % WG, d_t, :]

            for ch in range(nch):
                # --- uploads (program order per queue sets transfer order) ---
                # sync ring: x block 0 first, then the whole of W1 right behind
                xb0 = xpool.tile([128, DT * blocks[0][1]], BF16, tag="x0")
                nc.sync.dma_start(out=xb0, in_=x_d[0][ch])
                x_sb = [xb0]
                if ch == 0:
                    for g in range(NWG):
                        wg = w1pool.tile([128, WG, DT, 128], BF16, tag=f"w1g{g}")
                        nc.sync.dma_start(out=wg, in_=w1_d[g])
                        w1_sb.append(wg)
                # gpsimd ring: biases, then x blocks 1..nb-1
                if ch == 0:
                    nc.gpsimd.dma_start(out=b1_sb, in_=b1_d)
                    nc.gpsimd.dma_start(out=b2_sb, in_=b2_d)
                for bi in range(1, nb):
                    xb = xpool.tile([128, DT * blocks[bi][1]], BF16, tag=f"x{bi}")
                    nc.gpsimd.dma_start(out=xb, in_=x_d[bi][ch])
                    x_sb.append(xb)

                def xap(bi, d_t):
                    blk = blocks[bi][1]
                    return x_sb[bi][:, d_t * blk : (d_t + 1) * blk]

                ht_sb = hpool.tile([128, FT, rc], BF16, tag="ht")

                # W2 triggers (scalar ring) are interleaved into the A loops
                # once the W1/x traffic is done; one per slot below.
                w2_load = []
                if ch == 0 or nch > 1:
                    for o_t in range(OT):
                        w2t = w2pool.tile([128, FT, 128], BF16, tag="w2",
                                          name=f"w2_{ch}_{o_t}")
                        w2_load.append((o_t, w2t))

                def maybe_load_w2():
                    if w2_load:
                        o_t, w2t = w2_load.pop(0)
                        nc.scalar.dma_start(out=w2t, in_=w2_d[o_t])
                        w2_sb[o_t] = w2t

                # ---- phase A-0: first block only (starts after ~1MB DMA) ----
                b0, blk0 = blocks[0]
                for f_t in range(FT):
                    ps = pspool.tile([128, 512], F32, tag="ps")
                    for d_t in range(DT):
                        nc.tensor.matmul(
                            ps[:, :blk0],
                            lhsT=w1ap(f_t, d_t),
                            rhs=xap(0, d_t),
                            start=(d_t == 0),
                            stop=(d_t == DT - 1),
                        )
                    nc.scalar.activation(
                        out=ht_sb[:, f_t, b0 : b0 + blk0],
                        in_=ps[:, :blk0],
                        func=gelu,
                        bias=b1_sb[:, f_t : f_t + 1],
                        scale=1.0,
                    )
                    if nb == 1 and f_t >= 16:
                        maybe_load_w2()

                # ---- phase A-12: remaining blocks, stationary reused ----
                rest = list(range(1, nb))
                if rest:
                    for f_t in range(FT):
                        pss = [pspool.tile([128, 512], F32, tag="ps",
                                           name=f"psA{f_t}_{bi}") for bi in rest]
                        for d_t in range(DT):
                            for bi, ps in zip(rest, pss):
                                nc.tensor.matmul(
                                    ps[:, : blocks[bi][1]],
                                    lhsT=w1ap(f_t, d_t),
                                    rhs=xap(bi, d_t),
                                    start=(d_t == 0),
                                    stop=(d_t == DT - 1),
                                )
                        for bi, ps in zip(rest, pss):
                            rb0, rblk = blocks[bi]
                            nc.scalar.activation(
                                out=ht_sb[:, f_t, rb0 : rb0 + rblk],
                                in_=ps[:, :rblk],
                                func=gelu,
                                bias=b1_sb[:, f_t : f_t + 1],
                                scale=1.0,
                            )
                        maybe_load_w2()
                while w2_load:
                    maybe_load_w2()

                # ---- phase B: Y^T = W2^T-tiles @ H^T + b2 ----
                def drain(o_t, bi, ps):
                    bb0, bblk = blocks[bi]
                    yt_t = ypool.tile([128, 512], F32, tag="yt",
                                      name=f"yt{ch}_{o_t}_{bi}")
                    nc.scalar.activation(
                        out=yt_t[:, :bblk],
                        in_=ps[:, :bblk],
                        func=ident,
                        bias=b2_sb[:, o_t : o_t + 1],
                        scale=1.0,
                    )
                    nc.sync.dma_start(
                        out=yt_d[ch, o_t, :, bb0 : bb0 + bblk], in_=yt_t[:, :bblk]
                    )

                # order: big blocks first so the final drained block is small
                border = sorted(range(nb), key=lambda bi: -blocks[bi][1])
                for o_t in range(OT):
                    w2t = w2_sb[o_t]
                    last = o_t == OT - 1 and ch == nch - 1
                    if not last:
                        pss = [pspool.tile([128, 512], F32, tag="ps",
                                           name=f"psB{o_t}_{bi}") for bi in border]
                        for f_t in range(FT):
                            for bi, ps in zip(border, pss):
                                nc.tensor.matmul(
                                    ps[:, : blocks[bi][1]],
                                    lhsT=w2t[:, f_t, :],
                                    rhs=ht_sb[:, f_t, blocks[bi][0] : blocks[bi][0] + blocks[bi][1]],
                                    start=(f_t == 0),
                                    stop=(f_t == FT - 1),
                                )
                        for bi, ps in zip(border, pss):
                            drain(o_t, bi, ps)
                    else:
                        # final tile: sequential groups so drains overlap compute
                        for bi in border:
                            ps = pspool.tile([128, 512], F32, tag="ps",
                                             name=f"psBL_{bi}")
                            for f_t in range(FT):
                                nc.tensor.matmul(
                                    ps[:, : blocks[bi][1]],
                                    lhsT=w2t[:, f_t, :],
                                    rhs=ht_sb[:, f_t, blocks[bi][0] : blocks[bi][0] + blocks[bi][1]],
                                    start=(f_t == 0),
                                    stop=(f_t == FT - 1),
                                )
                            drain(o_t, bi, ps)

    nc.compile()
    _CACHE[key] = nc
    return nc


def _pick_shape(r_need: int):
    """Choose (rc, nch) given required per-core rows."""
    rc_max = 1456
    nch = 1
    while True:
        rc = -(-r_need // nch)          # ceil
        rc = max(256, -(-rc // 8) * 8)  # round up to 8, floor 256
        if rc <= rc_max:
            return rc, nch
        nch += 1


def kernel(inputs: np.ndarray, mask: np.ndarray, W1: np.ndarray, b1: np.ndarray,
           W2: np.ndarray, b2: np.ndarray) -> np.ndarray:
    global LAST_RESULTS
    B, T, Dm = inputs.shape
    assert Dm == D and W1.shape == (D, F) and W2.shape == (F, D)
    N = B * T

    x_flat = np.ascontiguousarray(np.asarray(inputs, dtype=np.float32).reshape(N, D))
    m_flat = np.asarray(mask).reshape(N).astype(bool)
    idx = np.flatnonzero(m_flat)
    na = idx.size
    out = np.zeros((N, D), dtype=np.float32)
    if na == 0:
        return out.reshape(B, T, D)

    r_need = -(-na // N_CORES)
    rc, nch = _pick_shape(r_need)
    cap = rc * nch

    nc = None
    while nc is None:
        try:
            nc = _build(rc, nch)
        except AssertionError:
            if nch >= 16:
                raise
            # SBUF overflow at this rc -> split into more chunks
            nch += 1
            rc = max(256, -(-(-(-r_need // nch)) // 8) * 8)
            cap = rc * nch

    blocks = _blocks(rc)
    idx_pad = np.zeros(N_CORES * cap, dtype=np.int64)
    idx_pad[:na] = idx
    xg = x_flat[idx_pad].astype(NP_BF16)  # [N_CORES*cap, D] bf16

    # weight/bias tilings (shared by all cores)
    # w1t[g, p, j, d_t, f] = W1[d_t*128+p, (g*WG+j)*128+f]
    w1t = np.ascontiguousarray(
        np.asarray(W1, np.float32).astype(NP_BF16)
        .reshape(DT, 128, NWG, WG, 128).transpose(2, 1, 3, 0, 4)
    )
    # w2t[o_t, p, f_t, o] = W2[f_t*128+p, o_t*128+o]
    w2t = np.ascontiguousarray(
        np.asarray(W2, np.float32).astype(NP_BF16)
        .reshape(FT, 128, OT, 128).transpose(2, 1, 0, 3)
    )
    b1m = np.ascontiguousarray(np.asarray(b1, np.float32).reshape(FT, 128).T)
    b2m = np.ascontiguousarray(np.asarray(b2, np.float32).reshape(OT, 128).T)

    in_maps = []
    for c in range(N_CORES):
        xc = xg[c * cap : (c + 1) * cap]  # [cap, D] bf16
        im = {"w1t": w1t, "b1m": b1m, "w2t": w2t, "b2m": b2m}
        # per-block x: xb[ch, p, d_t*blk + r] = xc[ch*rc + b0 + r, d_t*128 + p]
        for bi, (b0, blk) in enumerate(blocks):
            xb = np.empty((nch, 128, DT * blk), dtype=NP_BF16)
            for ch in range(nch):
                sl = xc[ch * rc + b0 : ch * rc + b0 + blk]  # [blk, D]
                xb[ch] = sl.reshape(blk, DT, 128).transpose(2, 1, 0).reshape(128, DT * blk)
            im[f"x{bi}"] = np.ascontiguousarray(xb)
        in_maps.append(im)

    _ensure_trace_support()
    res = bass_utils.run_bass_kernel_spmd(nc, in_maps, core_ids=list(range(N_CORES)))
    LAST_RESULTS = res

    y_parts = []
    for c in range(N_CORES):
        yt = res.results[c]["yt"]  # [nch, OT, 128, rc]
        for ch in range(nch):
            y_parts.append(yt[ch].reshape(D, rc).T)  # [rc, D]
    ycat = np.concatenate(y_parts, axis=0)  # [N_CORES*cap, D]
    out[idx] = ycat[:na]
    return out.reshape(B, T, D)


# revision 9
# speedup vs baseline: 1.0257x; 1.0257x over previous
"""Masked FFN kernel for trn2 (8 NeuronCores, SPMD data-parallel over rows).

Math: out = (gelu(x @ W1 + b1) @ W2 + b2) * mask  with masked-out rows exactly 0.

Strategy:
  - Host compacts the (B*T) rows down to the ~50% active ones (mask != 0),
    shards them evenly across 8 cores, pads per-core row count R to a
    block-friendly capacity.
  - All matmul operands are bf16 (fp32 PSUM accumulation, fp32 bias/act):
    halves DMA + SBUF traffic at the same PE rate; rel err ~4e-3.
  - Device computes the FFN on compacted rows only, in transposed layout:
      mm1: H^T[f, r] = sum_d W1[d, f] * X^T[d, r]   (W1 tile stationary)
      gelu+b1 fused on ScalarE (PSUM -> SBUF, bf16 out)
      mm2: Y^T[o, r] = sum_f W2[f, o] * H^T[f, r]   (W2 tile stationary)
      +b2 fused on ScalarE (PSUM -> SBUF fp32), DMA out.
  - Phase A covers all row blocks per f-tile (slow, safe W1 consumption);
    all input loads ride the gpsimd software-DGE ring ordered
    [x0, W1g0, biases, x1, x2, W1 rest, W2], which sustains delivery well
    ahead of consumption; y stores ride the sync ring.
  - PE warm-up matmuls on a memset tile cover the DVFS ramp during the
    initial DMA wait; the last output tile finishes as three sequential
    accumulation groups so its drains overlap compute.
"""

import numpy as np
import ml_dtypes

import concourse.tile as tile
from concourse import bacc, mybir
from concourse import bass_utils

N_CORES = 8
D = 1024       # model dim
F = 4096       # ffn dim
DT = D // 128  # 8 d-tiles
FT = F // 128  # 32 f-tiles
OT = D // 128  # 8 output tiles
WG = 2                 # W1 f-tiles per DMA group
NWG = FT // WG         # 16 groups
N_WARM = 14            # PE warm-up matmuls

F32 = mybir.dt.float32
BF16 = mybir.dt.bfloat16
NP_BF16 = ml_dtypes.bfloat16

_CACHE: dict = {}
LAST_RESULTS = None  # BassKernelResults of the most recent device run (for test harness)


def _ensure_trace_support():
    """If BASS_TRACE is set but the NTFF hook module is missing, install a
    local shim so run_bass_kernel_spmd's trace path works instead of crashing."""
    import os
    if not os.environ.get("BASS_TRACE"):
        return
    import sys, types
    try:
        import antenv.axon_hooks  # noqa: F401
    except ImportError:
        mod = types.ModuleType("antenv.axon_hooks")
        mod._h = None
        mod.set_axon_ntff_profile_hook = lambda h: setattr(mod, "_h", h)
        mod.get_axon_ntff_profile_hook = lambda: mod._h
        sys.modules["antenv.axon_hooks"] = mod
        try:
            from trn_agent_boot.trn_boot import _ntff_profile_via_ctypes
            mod.set_axon_ntff_profile_hook(
                _ntff_profile_via_ctypes("/opt/axon/libaxon_pjrt.so")
            )
        except Exception:
            pass
    try:
        bass_utils.upload_artifacts  # noqa: B018
        bass_utils.upload_artifacts = lambda tmpdir: tmpdir
    except Exception:
        pass


def _blocks(rc: int):
    """Row blocks: small first block (fast PE start), rest near-equal <= 512."""
    if rc <= 512:
        return [(0, rc)]
    rem = rc - 256
    n = -(-rem // 512)
    sizes = [rem // n + (1 if i < rem % n else 0) for i in range(n)]
    out = [(0, 256)]
    pos = 256
    for s in sizes:
        out.append((pos, s))
        pos += s
    return out


def _build(rc: int, nch: int):
    key = (rc, nch)
    if key in _CACHE:
        return _CACHE[key]

    nc = bacc.Bacc("TRN2", target_bir_lowering=False, debug=False, num_devices=N_CORES,
                   dynamic_dma_scratch_size=8192)
    blocks = _blocks(rc)
    nb = len(blocks)
    # one DRAM tensor per x block: per-partition fully contiguous
    x_d = [
        nc.dram_tensor(f"x{bi}", [nch, 128, DT * blk], BF16, kind="ExternalInput").ap()
        for bi, (b0, blk) in enumerate(blocks)
    ]
    w1_d = nc.dram_tensor("w1t", [NWG, 128, WG, DT, 128], BF16, kind="ExternalInput").ap()
    b1_d = nc.dram_tensor("b1m", [128, FT], F32, kind="ExternalInput").ap()
    w2_d = nc.dram_tensor("w2t", [OT, 128, FT, 128], BF16, kind="ExternalInput").ap()
    b2_d = nc.dram_tensor("b2m", [128, OT], F32, kind="ExternalInput").ap()
    yt_d = nc.dram_tensor("yt", [nch, OT, 128, rc], F32, kind="ExternalOutput").ap()

    gelu = mybir.ActivationFunctionType.Gelu_apprx_tanh
    ident = mybir.ActivationFunctionType.Identity

    with tile.TileContext(nc) as tc:
        with (
            tc.tile_pool(name="consts", bufs=1) as consts,
            tc.tile_pool(name="xpool", bufs=1) as xpool,
            tc.tile_pool(name="hpool", bufs=1) as hpool,
            tc.tile_pool(name="w1pool", bufs=1) as w1pool,
            tc.tile_pool(name="w2pool", bufs=3) as w2pool,
            tc.tile_pool(name="ypool", bufs=3) as ypool,
            tc.tile_pool(name="pspool", bufs=6, space="PSUM") as pspool,
            tc.tile_pool(name="warmps", bufs=1, space="PSUM") as warmps,
        ):
            b1_sb = consts.tile([128, FT], F32)
            b2_sb = consts.tile([128, OT], F32)

            # --- PE warm-up: ramp the clock while input DMAs are in flight ---
            warm_sb = consts.tile([128, 512], BF16)
            nc.vector.memset(warm_sb[:], 0.0)
            wps = warmps.tile([128, 512], F32)
            for _ in range(N_WARM):
                nc.tensor.matmul(wps[:], lhsT=warm_sb[:, :128], rhs=warm_sb[:],
                                 start=True, stop=True)

            w1_sb = []   # NWG group tiles, resident across chunks
            w2_sb = {}   # o_t -> rotating tile

            def w1ap(f_t, d_t):
                return w1_sb[f_t // WG][:, f_t % WG, d_t, :]

            for ch in range(nch):
                # --- all loads on the gpsimd ring; ring order = priority ---
                x_sb = []
                xb0 = xpool.tile([128, DT * blocks[0][1]], BF16, tag="x0")
                nc.gpsimd.dma_start(out=xb0, in_=x_d[0][ch])
                x_sb.append(xb0)
                if ch == 0:
                    wg0 = w1pool.tile([128, WG, DT, 128], BF16, tag="w1g0")
                    nc.gpsimd.dma_start(out=wg0, in_=w1_d[0])
                    w1_sb.append(wg0)
                    nc.gpsimd.dma_start(out=b1_sb, in_=b1_d)
                    nc.gpsimd.dma_start(out=b2_sb, in_=b2_d)
                for bi in range(1, nb):
                    xb = xpool.tile([128, DT * blocks[bi][1]], BF16, tag=f"x{bi}")
                    nc.gpsimd.dma_start(out=xb, in_=x_d[bi][ch])
                    x_sb.append(xb)
                if ch == 0:
                    for g in range(1, NWG):
                        wg = w1pool.tile([128, WG, DT, 128], BF16, tag=f"w1g{g}")
                        nc.gpsimd.dma_start(out=wg, in_=w1_d[g])
                        w1_sb.append(wg)
                if ch == 0 or nch > 1:
                    for o_t in range(OT):
                        w2t = w2pool.tile([128, FT, 128], BF16, tag="w2",
                                          name=f"w2_{ch}_{o_t}")
                        nc.gpsimd.dma_start(out=w2t, in_=w2_d[o_t])
                        w2_sb[o_t] = w2t

                def xap(bi, d_t):
                    blk = blocks[bi][1]
                    return x_sb[bi][:, d_t * blk : (d_t + 1) * blk]

                ht_sb = hpool.tile([128, FT, rc], BF16, tag="ht")

                # ---- phase A: H^T = gelu(W1^T-tiles @ X^T + b1), all blocks per f_t ----
                for f_t in range(FT):
                    pss = [pspool.tile([128, 512], F32, tag="ps",
                                       name=f"psA{f_t}_{bi}") for bi in range(nb)]
                    for d_t in range(DT):
                        for bi, ps in enumerate(pss):
                            nc.tensor.matmul(
                                ps[:, : blocks[bi][1]],
                                lhsT=w1ap(f_t, d_t),
                                rhs=xap(bi, d_t),
                                start=(d_t == 0),
                                stop=(d_t == DT - 1),
                            )
                    for bi, ps in enumerate(pss):
                        rb0, rblk = blocks[bi]
                        nc.scalar.activation(
                            out=ht_sb[:, f_t, rb0 : rb0 + rblk],
                            in_=ps[:, :rblk],
                            func=gelu,
                            bias=b1_sb[:, f_t : f_t + 1],
                            scale=1.0,
                        )

                # ---- phase B: Y^T = W2^T-tiles @ H^T + b2 ----
                def drain(o_t, bi, ps):
                    bb0, bblk = blocks[bi]
                    yt_t = ypool.tile([128, 512], F32, tag="yt",
                                      name=f"yt{ch}_{o_t}_{bi}")
                    nc.scalar.activation(
                        out=yt_t[:, :bblk],
                        in_=ps[:, :bblk],
                        func=ident,
                        bias=b2_sb[:, o_t : o_t + 1],
                        scale=1.0,
                    )
                    nc.sync.dma_start(
                        out=yt_d[ch, o_t, :, bb0 : bb0 + bblk], in_=yt_t[:, :bblk]
                    )

                # order: big blocks first so the final drained block is small
                border = sorted(range(nb), key=lambda bi: -blocks[bi][1])
                for o_t in range(OT):
                    w2t = w2_sb[o_t]
                    last = o_t == OT - 1 and ch == nch - 1
                    if not last:
                        pss = [pspool.tile([128, 512], F32, tag="ps",
                                           name=f"psB{o_t}_{bi}") for bi in border]
                        for f_t in range(FT):
                            for bi, ps in zip(border, pss):
                                nc.tensor.matmul(
                                    ps[:, : blocks[bi][1]],
                                    lhsT=w2t[:, f_t, :],
                                    rhs=ht_sb[:, f_t, blocks[bi][0] : blocks[bi][0] + blocks[bi][1]],
                                    start=(f_t == 0),
                                    stop=(f_t == FT - 1),
                                )
                        for bi, ps in zip(border, pss):
                            drain(o_t, bi, ps)
                    else:
                        # final tile: sequential groups so drains overlap compute
                        for bi in border:
                            ps = pspool.tile([128, 512], F32, tag="ps",
                                             name=f"psBL_{bi}")
                            for f_t in range(FT):
                                nc.tensor.matmul(
                                    ps[:, : blocks[bi][1]],
                                    lhsT=w2t[:, f_t, :],
                                    rhs=ht_sb[:, f_t, blocks[bi][0] : blocks[bi][0] + blocks[bi][1]],
                                    start=(f_t == 0),
                                    stop=(f_t == FT - 1),
                                )
                            drain(o_t, bi, ps)

    nc.compile()
    _CACHE[key] = nc
    return nc


def _pick_shape(r_need: int):
    """Choose (rc, nch) given required per-core rows."""
    rc_max = 1456
    nch = 1
    while True:
        rc = -(-r_need // nch)          # ceil
        rc = max(256, -(-rc // 8) * 8)  # round up to 8, floor 256
        if rc <= rc_max:
            return rc, nch
        nch += 1


def kernel(inputs: np.ndarray, mask: np.ndarray, W1: np.ndarray, b1: np.ndarray,
           W2: np.ndarray, b2: np.ndarray) -> np.ndarray:
    global LAST_RESULTS
    B, T, Dm = inputs.shape
    assert Dm == D and W1.shape == (D, F) and W2.shape == (F, D)
    N = B * T

    x_flat = np.ascontiguousarray(np.asarray(inputs, dtype=np.float32).reshape(N, D))
    m_flat = np.asarray(mask).reshape(N).astype(bool)
    idx = np.flatnonzero(m_flat)
    na = idx.size
    out = np.zeros((N, D), dtype=np.float32)
    if na == 0:
        return out.reshape(B, T, D)

    r_need = -(-na // N_CORES)
    rc, nch = _pick_shape(r_need)
    cap = rc * nch

    nc = None
    while nc is None:
        try:
            nc = _build(rc, nch)
        except AssertionError:
            if nch >= 16:
                raise
            # SBUF overflow at this rc -> split into more chunks
            nch += 1
            rc = max(256, -(-(-(-r_need // nch)) // 8) * 8)
            cap = rc * nch

    blocks = _blocks(rc)
    idx_pad = np.zeros(N_CORES * cap, dtype=np.int64)
    idx_pad[:na] = idx
    xg = x_flat[idx_pad].astype(NP_BF16)  # [N_CORES*cap, D] bf16

    # weight/bias tilings (shared by all cores)
    # w1t[g, p, j, d_t, f] = W1[d_t*128+p, (g*WG+j)*128+f]
    w1t = np.ascontiguousarray(
        np.asarray(W1, np.float32).astype(NP_BF16)
        .reshape(DT, 128, NWG, WG, 128).transpose(2, 1, 3, 0, 4)
    )
    # w2t[o_t, p, f_t, o] = W2[f_t*128+p, o_t*128+o]
    w2t = np.ascontiguousarray(
        np.asarray(W2, np.float32).astype(NP_BF16)
        .reshape(FT, 128, OT, 128).transpose(2, 1, 0, 3)
    )
    b1m = np.ascontiguousarray(np.asarray(b1, np.float32).reshape(FT, 128).T)
    b2m = np.ascontiguousarray(np.asarray(b2, np.float32).reshape(OT, 128).T)

    in_maps = []
    for c in range(N_CORES):
        xc = xg[c * cap : (c + 1) * cap]  # [cap, D] bf16
        im = {"w1t": w1t, "b1m": b1m, "w2t": w2t, "b2m": b2m}
        # per-block x: xb[ch, p, d_t*blk + r] = xc[ch*rc + b0 + r, d_t*128 + p]
        for bi, (b0, blk) in enumerate(blocks):
            xb = np.empty((nch, 128, DT * blk), dtype=NP_BF16)
            for ch in range(nch):
                sl = xc[ch * rc + b0 : ch * rc + b0 + blk]  # [blk, D]
                xb[ch] = sl.reshape(blk, DT, 128).transpose(2, 1, 0).reshape(128, DT * blk)
            im[f"x{bi}"] = np.ascontiguousarray(xb)
        in_maps.append(im)

    _ensure_trace_support()
    res = bass_utils.run_bass_kernel_spmd(nc, in_maps, core_ids=list(range(N_CORES)))
    LAST_RESULTS = res

    y_parts = []
    for c in range(N_CORES):
        yt = res.results[c]["yt"]  # [nch, OT, 128, rc]
        for ch in range(nch):
            y_parts.append(yt[ch].reshape(D, rc).T)  # [rc, D]
    ycat = np.concatenate(y_parts, axis=0)  # [N_CORES*cap, D]
    out[idx] = ycat[:na]
    return out.reshape(B, T, D)
